# revision 1
# baseline (speedup 1.0000x reference)
"""Trainium2 Bass kernel for nn_DecoderBlockBVL (B=2,V=8,L=256,C=768,H=12).

Sharding: 8 cores; core c handles batch b=c//4, query-slice g=c%4
(rows [g*64,(g+1)*64) of every view). Phase 1 (per-view self-attn) is
computed redundantly for the whole batch on each core (phase 2 needs
k/v for all tokens); phase-2 queries and the MLP cover only the core's
512 tokens. The host permutes each view's rows so the core's slice
sits at the front -> every core runs one identical SPMD program.

Layouts: residual stream token-major [tok, C]; matmul operands
feature-major (x^T) via PE transpose after each LN; weights
pre-transposed on the host to [C_in, F]. Matmuls run in float32r
(full PE rate at moving dim >= 256); attention probs/V and the fc2
operands in bf16.
"""

import numpy as np
import ml_dtypes

import concourse.bass as bass
import concourse.bacc as bacc
import concourse.mybir as mybir
import concourse.tile as tile
from concourse.bass_utils import run_bass_kernel_spmd
from concourse.masks import make_identity

dt = mybir.dt
F32 = dt.float32
F32R = dt.float32r
BF16 = dt.bfloat16
AF = mybir.ActivationFunctionType
ALU = mybir.AluOpType

B, V, L, C, H = 2, 8, 256, 768, 12
HD = C // H          # 64
S = V * L            # 2048
HID = 3072
NCORES = 8
G = 4                # cores per batch
QS = L // G          # 64 queries per view per core
MYQ = V * QS         # 512 tokens per core
SCALE = HD ** -0.5
CK = C // 128        # 6
SK = S // 128        # 16
HK = HID // 128      # 24
NHALF = ((0, 384), (384, 384))


def _kr(v):
    """allowed key prefix length for query view v (block-causal mask)"""
    return 512 if v < 2 else 256 * (v + 1)


def _build(ln_identity: bool, zero_bias: bool, sim_gelu: bool = False):
    nc = bacc.Bacc()

    xb = nc.declare_dram_parameter("xb", [S, C], F32, isOutput=False)
    wqkv = nc.declare_dram_parameter("wqkv_t", [C, 3 * C], BF16, isOutput=False)
    wproj = nc.declare_dram_parameter("wproj_t", [C, C], BF16, isOutput=False)
    wq = nc.declare_dram_parameter("wq_t", [C, C], BF16, isOutput=False)
    wk = nc.declare_dram_parameter("wk_t", [C, C], BF16, isOutput=False)
    wv = nc.declare_dram_parameter("wv_t", [C, C], BF16, isOutput=False)
    wcp = nc.declare_dram_parameter("wcproj_t", [C, C], BF16, isOutput=False)
    wf1 = nc.declare_dram_parameter("wfc1_t", [C, HID], BF16, isOutput=False)
    wf2 = nc.declare_dram_parameter("wfc2_t", [HID, C], BF16, isOutput=False)
    out = nc.declare_dram_parameter("out", [MYQ, C], F32, isOutput=True)

    lng = lnb = bias = f1b = None
    if not ln_identity:
        lng = nc.declare_dram_parameter("ln_g", [3, C], F32, isOutput=False)
        lnb = nc.declare_dram_parameter("ln_b", [3, C], F32, isOutput=False)
    if not zero_bias:
        bias = nc.declare_dram_parameter("bias3", [3, C], F32, isOutput=False)
        f1b = nc.declare_dram_parameter("fc1_b", [HID], F32, isOutput=False)

    x1d = nc.dram_tensor("x1d", [S, C], F32)  # phase-1 output spill

    with tile.TileContext(nc) as tc, \
         tc.tile_pool(name="consts", bufs=1) as consts:
        identb = consts.tile([128, 128], BF16)
        make_identity(nc, identb)
        eps = consts.tile([128, 1], F32)
        nc.vector.memset(eps, 1e-5)

        gbt = bbt = bias_bc = f1b_t = None
        if not ln_identity:
            gbt = consts.tile([128, 3, C], F32)
            bbt = consts.tile([128, 3, C], F32)
            for t, src in ((gbt, lng), (bbt, lnb)):
                bc = bass.AP(tensor=src.tensor, offset=src.offset,
                             ap=[[0, 128]] + list(src.ap))
                nc.gpsimd.dma_start(out=t[:], in_=bc)
        if not zero_bias:
            bias_bc = consts.tile([128, 3, C], F32)
            bc = bass.AP(tensor=bias.tensor, offset=bias.offset,
                         ap=[[0, 128]] + list(bias.ap))
            nc.gpsimd.dma_start(out=bias_bc[:], in_=bc)
            f1b_t = consts.tile([128, HK], F32)
            nc.gpsimd.dma_start(out=f1b_t[:], in_=f1b.rearrange("(a p) -> p a", p=128))

        def ln(pool, x_ap, h_ap, which):
            """layernorm over free dim C; x_ap/h_ap [128, C]"""
            st = pool.tile([128, 3, 6], F32, tag="ln_st")
            for sg in range(3):
                nc.vector.bn_stats(out=st[:, sg, :],
                                   in_=x_ap[:, sg * 256:(sg + 1) * 256])
            mv = pool.tile([128, 2], F32, tag="ln_mv")
            nc.vector.bn_aggr(out=mv[:], in_=st[:])
            nm = pool.tile([128, 2], F32, tag="ln_nm")  # [neg-mean, rstd]
            nc.vector.tensor_scalar_mul(nm[:, 0:1], mv[:, 0:1], -1.0)
            nc.scalar.activation(nm[:, 1:2], mv[:, 1:2], AF.Sqrt, bias=eps[:])
            nc.vector.reciprocal(nm[:, 1:2], nm[:, 1:2])
            nc.vector.tensor_scalar(h_ap, x_ap, nm[:, 0:1], nm[:, 1:2],
                                    ALU.add, ALU.mult)
            if not ln_identity:
                nc.vector.tensor_mul(h_ap, h_ap, gbt[:, which, :])
                nc.vector.tensor_add(h_ap, h_ap, bbt[:, which, :])

        def transpose_cols(psp, src, dst, j, n):
            """n [128,128] bf16 blocks src(mc) -> dst[:, j, :] ([128, n*128])"""
            ps = psp.tile([128, n * 128], BF16, tag="scb")
            for mc in range(n):
                nc.tensor.matmul(ps[:, mc * 128:(mc + 1) * 128], src(mc),
                                 identb[:], is_transpose=True)
            nc.any.tensor_copy(dst[:, j, :], ps[:])

        # =================== phase 1: per-view self-attention ===================
        with tc.tile_pool(name="p1w", bufs=1) as p1w, \
             tc.tile_pool(name="p1b", bufs=2) as p1b, \
             tc.tile_pool(name="p1s", bufs=3) as p1s, \
             tc.tile_pool(name="ps_sc", bufs=2, space="PSUM") as ps_sc, \
             tc.tile_pool(name="ps_mb", bufs=2, space="PSUM") as ps_mb, \
             tc.tile_pool(name="ps_o", bufs=2, space="PSUM") as ps_o:

            wqkv_s = p1w.tile([128, CK, 3 * C], BF16)
            wproj_s = p1w.tile([128, CK, C], BF16)
            for kc in range(CK):
                nc.sync.dma_start(out=wqkv_s[:, kc, :], in_=wqkv[kc * 128:(kc + 1) * 128, :])
                nc.sync.dma_start(out=wproj_s[:, kc, :], in_=wproj[kc * 128:(kc + 1) * 128, :])

            for v in range(V):
                xv = p1b.tile([128, 2, C], F32, tag="xv")
                for mc in range(2):
                    nc.sync.dma_start(out=xv[:, mc, :],
                                      in_=xb[v * L + mc * 128: v * L + (mc + 1) * 128, :])
                h1 = p1b.tile([128, 2, C], BF16, tag="h1")
                for mc in range(2):
                    ln(p1s, xv[:, mc, :], h1[:, mc, :], 0)
                h1T = p1b.tile([128, CK, 256], BF16, tag="h1T")
                for j in range(CK):
                    transpose_cols(ps_sc, lambda mc: h1[:, mc, j * 128:(j + 1) * 128],
                                   h1T, j, 2)

                # q^T,k^T feature-major [1536, 256]
                qkT = p1b.tile([128, 12, 256], BF16, tag="qkT")
                for mo in range(12):
                    ps = ps_sc.tile([128, 256], F32, tag="sc")
                    for kc in range(CK):
                        nc.tensor.matmul(ps[:], wqkv_s[:, kc, mo * 128:(mo + 1) * 128],
                                         h1T[:, kc, :], start=kc == 0, stop=kc == CK - 1)
                    nc.any.tensor_copy(qkT[:, mo, :], ps[:])
                # v token-major bf16 [256, 768]
                v1 = p1b.tile([128, 2, C], BF16, tag="v1")
                for mt in range(2):
                    pss = [ps_mb.tile([128, 384], F32, tag="mb", name=f"mbh{i}") for i in range(2)]
                    for kc in range(CK):
                        for i, (no, nn_) in enumerate(NHALF):
                            nc.tensor.matmul(pss[i][:],
                                             h1T[:, kc, mt * 128:(mt + 1) * 128],
                                             wqkv_s[:, kc, 2 * C + no:2 * C + no + nn_],
                                             start=kc == 0, stop=kc == CK - 1)
                    for i, (no, nn_) in enumerate(NHALF):
                        nc.any.tensor_copy(v1[:, mt, no:no + nn_], pss[i][:])

                o1T = p1b.tile([128, CK, 256], BF16, tag="o1T")
                for hp in range(6):
                    ops = ps_o.tile([128, 256], F32, tag="o")
                    for hh in range(2):
                        h = hp * 2 + hh
                        qh = qkT[hh * 64:(hh + 1) * 64, hp, :]       # [64, 256]
                        kh = qkT[hh * 64:(hh + 1) * 64, 6 + hp, :]   # [64, 256]
                        sps = ps_sc.tile([128, 2, 256], F32, tag="sc")
                        for mc in range(2):
                            nc.tensor.matmul(sps[:, mc, :], qh[:, mc * 128:(mc + 1) * 128],
                                             kh, start=True, stop=True)
                        probs = p1s.tile([128, 2, 256], BF16, tag="probs")
                        sums = p1s.tile([128, 2], F32, tag="sums")
                        for mc in range(2):
                            nc.scalar.activation(probs[:, mc, :], sps[:, mc, :], AF.Exp,
                                                 scale=SCALE, accum_out=sums[:, mc:mc + 1])
                        nc.vector.reciprocal(sums[:], sums[:])
                        for mc in range(2):
                            nc.vector.tensor_scalar_mul(probs[:, mc, :], probs[:, mc, :],
                                                        sums[:, mc:mc + 1])
                        pTps = ps_sc.tile([128, 2, 256], BF16, tag="scb")
                        for kb in range(2):
                            for mc in range(2):
                                nc.tensor.matmul(pTps[:, kb, mc * 128:(mc + 1) * 128],
                                                 probs[:, mc, kb * 128:(kb + 1) * 128],
                                                 identb[:], is_transpose=True)
                        pT = p1s.tile([128, 2, 256], BF16, tag="pT")
                        for kb in range(2):
                            nc.any.tensor_copy(pT[:, kb, :], pTps[:, kb, :])
                        for kb in range(2):
                            nc.tensor.matmul(ops[hh * 64:(hh + 1) * 64, :],
                                             v1[:, kb, h * 64:(h + 1) * 64], pT[:, kb, :],
                                             start=kb == 0, stop=kb == 1)
                    nc.any.tensor_copy(o1T[:, hp, :], ops[:])

                # proj + residual -> x1 (token-major), spill to DRAM
                for mt in range(2):
                    pss = [ps_mb.tile([128, 384], F32, tag="mb", name=f"mbh{i}") for i in range(2)]
                    for kc in range(CK):
                        for i, (no, nn_) in enumerate(NHALF):
                            nc.tensor.matmul(pss[i][:],
                                             o1T[:, kc, mt * 128:(mt + 1) * 128],
                                             wproj_s[:, kc, no:no + nn_],
                                             start=kc == 0, stop=kc == CK - 1)
                    x1v = p1b.tile([128, C], F32, tag="x1v")
                    for i, (no, nn_) in enumerate(NHALF):
                        nc.vector.tensor_add(x1v[:, no:no + nn_], pss[i][:],
                                             xv[:, mt, no:no + nn_])
                    if not zero_bias:
                        nc.vector.tensor_add(x1v[:], x1v[:], bias_bc[:, 0, :])
                    nc.sync.dma_start(out=x1d[v * L + mt * 128: v * L + (mt + 1) * 128, :],
                                      in_=x1v[:])

        # =================== phase 2 + 3 ===================
        with tc.tile_pool(name="p23", bufs=1) as p23:
            x2 = p23.tile([128, 4, C], F32)

            with tc.tile_pool(name="p2p", bufs=1) as p2p:
                k2T = p2p.tile([128, CK, S], BF16)
                v2 = p2p.tile([128, SK, C], BF16)
                h2mT = p2p.tile([128, CK, MYQ], BF16)
                q2T = p2p.tile([128, CK, MYQ], BF16)
                o2T = p2p.tile([128, CK, MYQ], BF16)

                # --- 2a: ln2 + k/v projections, streamed per view ---
                with tc.tile_pool(name="p2aw", bufs=1) as p2aw, \
                     tc.tile_pool(name="p2ab", bufs=2) as p2ab, \
                     tc.tile_pool(name="p2as", bufs=3) as p2as, \
                     tc.tile_pool(name="ps2_sc", bufs=2, space="PSUM") as ps2_sc, \
                     tc.tile_pool(name="ps2_mb", bufs=3, space="PSUM") as ps2_mb:
                    wk_s = p2aw.tile([128, CK, C], BF16)
                    wv_s = p2aw.tile([128, CK, C], BF16)
                    for kc in range(CK):
                        nc.sync.dma_start(out=wk_s[:, kc, :], in_=wk[kc * 128:(kc + 1) * 128, :])
                        nc.sync.dma_start(out=wv_s[:, kc, :], in_=wv[kc * 128:(kc + 1) * 128, :])

                    for v in range(V):
                        x1v = p2ab.tile([128, 2, C], F32, tag="x1v")
                        for mc in range(2):
                            nc.sync.dma_start(
                                out=x1v[:, mc, :],
                                in_=x1d[v * L + mc * 128: v * L + (mc + 1) * 128, :])
                        h2 = p2ab.tile([128, 2, C], BF16, tag="h2")
                        for mc in range(2):
                            ln(p2as, x1v[:, mc, :], h2[:, mc, :], 1)
                        h2T = p2ab.tile([128, CK, 256], BF16, tag="h2T")
                        for j in range(CK):
                            transpose_cols(ps2_sc,
                                           lambda mc: h2[:, mc, j * 128:(j + 1) * 128],
                                           h2T, j, 2)
                        for kc in range(CK):
                            nc.any.tensor_copy(h2mT[:, kc, v * QS:(v + 1) * QS],
                                               h2T[:, kc, 0:QS])
                        for mo in range(CK):
                            ps = ps2_sc.tile([128, 256], F32, tag="sc")
                            for kc in range(CK):
                                nc.tensor.matmul(ps[:],
                                                 wk_s[:, kc, mo * 128:(mo + 1) * 128],
                                                 h2T[:, kc, :],
                                                 start=kc == 0, stop=kc == CK - 1)
                            nc.any.tensor_copy(k2T[:, mo, v * L:(v + 1) * L], ps[:])
                        for mt in range(2):
                            pss = [ps2_mb.tile([128, 384], F32, tag="mb", name=f"mbh{i}") for i in range(2)]
                            for kc in range(CK):
                                for i, (no, nn_) in enumerate(NHALF):
                                    nc.tensor.matmul(pss[i][:],
                                                     h2T[:, kc, mt * 128:(mt + 1) * 128],
                                                     wv_s[:, kc, no:no + nn_],
                                                     start=kc == 0, stop=kc == CK - 1)
                            for i, (no, nn_) in enumerate(NHALF):
                                nc.any.tensor_copy(v2[:, v * 2 + mt, no:no + nn_], pss[i][:])

                # --- q projection for my 512 tokens ---
                with tc.tile_pool(name="p2qw", bufs=1) as p2qw, \
                     tc.tile_pool(name="ps2q", bufs=2, space="PSUM") as ps2q:
                    wq_s = p2qw.tile([128, CK, C], BF16)
                    for kc in range(CK):
                        nc.sync.dma_start(out=wq_s[:, kc, :], in_=wq[kc * 128:(kc + 1) * 128, :])
                    for mo in range(CK):
                        ps = ps2q.tile([128, MYQ], F32)
                        for kc in range(CK):
                            nc.tensor.matmul(ps[:], wq_s[:, kc, mo * 128:(mo + 1) * 128],
                                             h2mT[:, kc, :],
                                             start=kc == 0, stop=kc == CK - 1)
                        nc.any.tensor_copy(q2T[:, mo, :], ps[:])

                # --- 2b: block-causal attention over key prefixes ---
                with tc.tile_pool(name="p2bs", bufs=3) as p2bs, \
                     tc.tile_pool(name="ps2b_sc", bufs=3, space="PSUM") as ps2b_sc, \
                     tc.tile_pool(name="ps2b_o", bufs=2, space="PSUM") as ps2b_o:
                    for hp in range(6):
                        ops = ps2b_o.tile([128, MYQ], F32, tag="o")
                        for hh in range(2):
                            h = hp * 2 + hh
                            for v in range(V):
                                kr = _kr(v)
                                nk = (kr + 511) // 512
                                nkb = kr // 128
                                qh = q2T[hh * 64:(hh + 1) * 64, hp, v * QS:(v + 1) * QS]
                                probs = p2bs.tile([64, S], BF16, tag="probs2")
                                sums = p2bs.tile([64, 4], F32, tag="sums2")
                                for ck in range(nk):
                                    kw = min(512, kr - ck * 512)
                                    sps = ps2b_sc.tile([64, 512], F32, tag="sc")
                                    nc.tensor.matmul(sps[:, :kw], qh,
                                                     k2T[hh * 64:(hh + 1) * 64, hp,
                                                         ck * 512:ck * 512 + kw],
                                                     start=True, stop=True)
                                    nc.scalar.activation(probs[:, ck * 512:ck * 512 + kw],
                                                         sps[:, :kw], AF.Exp, scale=SCALE,
                                                         accum_out=sums[:, ck:ck + 1])
                                rtot = p2bs.tile([64, 1], F32, tag="rtot")
                                nc.vector.reduce_sum(out=rtot[:], in_=sums[:, 0:nk],
                                                     axis=mybir.AxisListType.X)
                                nc.vector.reciprocal(rtot[:], rtot[:])
                                nc.vector.tensor_scalar_mul(probs[:, :kr], probs[:, :kr],
                                                            rtot[:])
                                pT = p2bs.tile([128, SK, QS], BF16, tag="pT2")
                                for g4 in range((nkb + 3) // 4):
                                    nb = min(4, nkb - g4 * 4)
                                    pTps = ps2b_sc.tile([128, 4, QS], BF16, tag="scb")
                                    for i in range(nb):
                                        kb = g4 * 4 + i
                                        nc.tensor.matmul(pTps[:, i, :],
                                                         probs[:, kb * 128:(kb + 1) * 128],
                                                         identb[0:64, 0:QS],
                                                         is_transpose=True)
                                    nc.any.tensor_copy(pT[:, g4 * 4:g4 * 4 + nb, :],
                                                       pTps[:, 0:nb, :])
                                for kb in range(nkb):
                                    nc.tensor.matmul(
                                        ops[hh * 64:(hh + 1) * 64, v * QS:(v + 1) * QS],
                                        v2[:, kb, h * 64:(h + 1) * 64], pT[:, kb, :],
                                        start=kb == 0, stop=kb == nkb - 1)
                        nc.any.tensor_copy(o2T[:, hp, :], ops[:])

                # --- 2c: cproj + residual ---
                with tc.tile_pool(name="p2cw", bufs=1) as p2cw, \
                     tc.tile_pool(name="p2cs", bufs=2) as p2cs, \
                     tc.tile_pool(name="ps2c", bufs=3, space="PSUM") as ps2c:
                    wcp_s = p2cw.tile([128, CK, C], BF16)
                    for kc in range(CK):
                        nc.sync.dma_start(out=wcp_s[:, kc, :],
                                          in_=wcp[kc * 128:(kc + 1) * 128, :])
                    x1m = p2cw.tile([128, 4, C], F32)
                    for v in range(V):
                        nc.sync.dma_start(out=x1m[(v % 2) * 64:(v % 2) * 64 + 64, v // 2, :],
                                          in_=x1d[v * L: v * L + QS, :])
                    for mt in range(4):
                        pss = [ps2c.tile([128, 384], F32, tag="mb", name=f"mbh{i}") for i in range(2)]
                        for kc in range(CK):
                            for i, (no, nn_) in enumerate(NHALF):
                                nc.tensor.matmul(pss[i][:],
                                                 o2T[:, kc, mt * 128:(mt + 1) * 128],
                                                 wcp_s[:, kc, no:no + nn_],
                                                 start=kc == 0, stop=kc == CK - 1)
                        for i, (no, nn_) in enumerate(NHALF):
                            nc.vector.tensor_add(x2[:, mt, no:no + nn_], pss[i][:],
                                                 x1m[:, mt, no:no + nn_])
                        if not zero_bias:
                            nc.vector.tensor_add(x2[:, mt, :], x2[:, mt, :],
                                                 bias_bc[:, 1, :])

            # =================== phase 3: MLP ===================
            with tc.tile_pool(name="p3w", bufs=1) as p3w, \
                 tc.tile_pool(name="p3one", bufs=1) as p3one, \
                 tc.tile_pool(name="p3s", bufs=3) as p3s, \
                 tc.tile_pool(name="ps3_sc", bufs=2, space="PSUM") as ps3_sc, \
                 tc.tile_pool(name="ps3_mb", bufs=3, space="PSUM") as ps3_mb:
                wf1_s = p3w.tile([128, CK, HID], BF16)
                for kc in range(CK):
                    nc.sync.dma_start(out=wf1_s[:, kc, :], in_=wf1[kc * 128:(kc + 1) * 128, :])
                wf2_s = p3w.tile([128, HK, C], BF16)
                for kc in range(HK):
                    nc.sync.dma_start(out=wf2_s[:, kc, :], in_=wf2[kc * 128:(kc + 1) * 128, :])

                h3 = p3one.tile([128, 4, C], BF16)
                for mt in range(4):
                    ln(p3s, x2[:, mt, :], h3[:, mt, :], 2)
                h3T = p3one.tile([128, CK, MYQ], BF16)
                for j in range(CK):
                    transpose_cols(ps3_sc, lambda mc: h3[:, mc, j * 128:(j + 1) * 128],
                                   h3T, j, 4)
                g1T = p3one.tile([128, HK, MYQ], BF16)
                for mo in range(HK):
                    ps = ps3_sc.tile([128, MYQ], F32, tag="sc")
                    for kc in range(CK):
                        nc.tensor.matmul(ps[:], wf1_s[:, kc, mo * 128:(mo + 1) * 128],
                                         h3T[:, kc, :], start=kc == 0, stop=kc == CK - 1)
                    if sim_gelu:
                        # tanh-approx gelu from sim-supported ops (sim only)
                        xg = p3s.tile([128, MYQ], F32, tag="xg")
                        if zero_bias:
                            nc.any.tensor_copy(xg[:], ps[:])
                        else:
                            nc.scalar.activation(xg[:], ps[:], AF.Identity,
                                                 bias=f1b_t[:, mo:mo + 1])
                        x2g = p3s.tile([128, MYQ], F32, tag="x2g")
                        nc.scalar.activation(x2g[:], xg[:], AF.Square)
                        nc.vector.tensor_scalar(x2g[:], x2g[:], 0.0356774081,
                                                0.7978845608, ALU.mult, ALU.add)
                        nc.vector.tensor_mul(x2g[:], x2g[:], xg[:])
                        nc.scalar.activation(x2g[:], x2g[:], AF.Tanh)
                        nc.vector.tensor_mul(x2g[:], x2g[:], xg[:])
                        nc.vector.tensor_add(x2g[:], x2g[:], xg[:])
                        nc.vector.tensor_scalar_mul(x2g[:], x2g[:], 0.5)
                        nc.any.tensor_copy(g1T[:, mo, :], x2g[:])
                    elif zero_bias:
                        nc.scalar.activation(g1T[:, mo, :], ps[:], AF.Gelu)
                    else:
                        nc.scalar.activation(g1T[:, mo, :], ps[:], AF.Gelu,
                                             bias=f1b_t[:, mo:mo + 1])
                for mt in range(4):
                    pss = [ps3_mb.tile([128, 384], F32, tag="mb", name=f"mbh{i}") for i in range(2)]
                    for kc in range(HK):
                        for i, (no, nn_) in enumerate(NHALF):
                            nc.tensor.matmul(pss[i][:],
                                             g1T[:, kc, mt * 128:(mt + 1) * 128],
                                             wf2_s[:, kc, no:no + nn_],
                                             start=kc == 0, stop=kc == HK - 1)
                    yo = p3s.tile([128, C], F32, tag="yo")
                    for i, (no, nn_) in enumerate(NHALF):
                        nc.vector.tensor_add(yo[:, no:no + nn_], pss[i][:],
                                             x2[:, mt, no:no + nn_])
                    if not zero_bias:
                        nc.vector.tensor_add(yo[:], yo[:], bias_bc[:, 2, :])
                    nc.sync.dma_start(out=out[mt * 128:(mt + 1) * 128, :], in_=yo[:])

    nc.finalize()
    return nc


_CACHE = {}


def _get_nc(ln_identity, zero_bias, sim_gelu=False):
    key = (ln_identity, zero_bias, sim_gelu)
    if key not in _CACHE:
        _CACHE[key] = _build(ln_identity, zero_bias, sim_gelu)
    return _CACHE[key]


def _prep_inputs(inputs):
    x = np.asarray(inputs["x"], np.float32)          # [B, V, L, C]
    ln_identity = all(np.all(np.asarray(inputs[f"ln{i}_g"]) == 1.0)
                      and np.all(np.asarray(inputs[f"ln{i}_b"]) == 0.0)
                      for i in (1, 2, 3))
    zero_bias = all(np.all(np.asarray(inputs[k]) == 0.0)
                    for k in ("attn_proj_b", "cproj_b", "fc1_b", "fc2_b"))

    tr = lambda k: np.ascontiguousarray(
        np.asarray(inputs[k], np.float32).T).astype(ml_dtypes.bfloat16)
    wqkv_t, wproj_t = tr("qkv_w"), tr("attn_proj_w")
    wq_t, wk_t, wv_t, wcp_t = tr("q_w"), tr("k_w"), tr("v_w"), tr("cproj_w")
    wf1_t = tr("fc1_w")
    wf2_t = tr("fc2_w")

    in_maps = []
    for c in range(NCORES):
        b, g = divmod(c, G)
        xbp = np.empty((S, C), np.float32)
        for v in range(V):
            xv = x[b, v]
            xbp[v * L: v * L + QS] = xv[g * QS:(g + 1) * QS]
            xbp[v * L + QS: v * L + QS + g * QS] = xv[0: g * QS]
            xbp[v * L + QS + g * QS: (v + 1) * L] = xv[(g + 1) * QS:]
        m = {"xb": xbp, "wqkv_t": wqkv_t, "wproj_t": wproj_t, "wq_t": wq_t,
             "wk_t": wk_t, "wv_t": wv_t, "wcproj_t": wcp_t, "wfc1_t": wf1_t,
             "wfc2_t": wf2_t}
        if not ln_identity:
            m["ln_g"] = np.stack([np.asarray(inputs[f"ln{i}_g"], np.float32)
                                  for i in (1, 2, 3)])
            m["ln_b"] = np.stack([np.asarray(inputs[f"ln{i}_b"], np.float32)
                                  for i in (1, 2, 3)])
        if not zero_bias:
            m["bias3"] = np.stack([np.asarray(inputs["attn_proj_b"], np.float32),
                                   np.asarray(inputs["cproj_b"], np.float32),
                                   np.asarray(inputs["fc2_b"], np.float32)])
            m["fc1_b"] = np.asarray(inputs["fc1_b"], np.float32)
        in_maps.append(m)
    return in_maps, ln_identity, zero_bias


def _assemble(results):
    out = np.empty((B, V, L, C), np.float32)
    for c in range(NCORES):
        b, g = divmod(c, G)
        oc = np.asarray(results[c]["out"])
        for v in range(V):
            out[b, v, g * QS:(g + 1) * QS] = oc[v * QS:(v + 1) * QS]
    return out


def kernel(**inputs):
    in_maps, ln_identity, zero_bias = _prep_inputs(inputs)
    nc = _get_nc(ln_identity, zero_bias)
    res = run_bass_kernel_spmd(nc, in_maps, core_ids=list(range(NCORES)))
    return _assemble(res.results)



# revision 40
# speedup vs baseline: 137.8913x; 137.8913x over previous
"""Trainium2 Bass kernel for nn_DecoderBlockBVL (B=2,V=8,L=256,C=768,H=12).

Sharding (comm version): 8 cores in two groups of 4 (one per batch).
Core with group-rank g owns views (2g, 2g+1): it runs phase 1 and the
k/v projections for those views only. An AllToAll hands every core the
x1 rows of ITS query slice (rows [g*64,(g+1)*64) of every view) and an
AllGather shares the k/v projections of all 2048 tokens. Phase-2
queries and the MLP cover only the core's 512 tokens.

Kernel design notes:
- Attention computes scores TRANSPOSED (keys on partitions) so the
  probabilities come out as [key, query] tiles that feed PV directly as
  the stationary operand -- no PE transposes of probabilities.
- QK^T runs as two concurrent 64-row tile_position row-tiles.
- PV is token-major with a ones-column appended to V, so the softmax
  denominator lands in column 64 of the accumulator and normalization
  is a native per-partition reciprocal+multiply.
- Dense projections use 512-wide moving operands.
"""

import contextlib

import numpy as np
import ml_dtypes

import concourse.bass as bass
import concourse.bacc as bacc
import concourse.mybir as mybir
import concourse.tile as tile
from concourse.bass_utils import run_bass_kernel_spmd
from concourse.masks import make_identity

dt = mybir.dt
F32 = dt.float32
BF16 = dt.bfloat16
AF = mybir.ActivationFunctionType
ALU = mybir.AluOpType

B, V, L, C, H = 2, 8, 256, 768, 12
HD = C // H          # 64
S = V * L            # 2048
HID = 3072
NCORES = 8
G = 4                # cores per batch
QS = L // G          # 64 queries per view per core
MYQ = V * QS         # 512 tokens per core
SCALE = HD ** -0.5
CK = C // 128        # 6
SK = S // 128        # 16
HK = HID // 128      # 24
NP = 6               # head pairs
RG = [[0, 1, 2, 3], [4, 5, 6, 7]]

USE_COMM = True


def _build(ln_identity: bool, zero_bias: bool, sim_gelu: bool = False,
           comm: bool = USE_COMM):
    nc = bacc.Bacc()

    npair = 1 if comm else 4
    xb = nc.declare_dram_parameter("xb", [npair * 512, C], F32, isOutput=False)
    wqkv = nc.declare_dram_parameter("wqkv_t", [C, 3 * C], BF16, isOutput=False)
    wproj = nc.declare_dram_parameter("wproj_t", [C, C], BF16, isOutput=False)
    wq = nc.declare_dram_parameter("wq_t", [C, C], BF16, isOutput=False)
    wk = nc.declare_dram_parameter("wk_t", [C, C], BF16, isOutput=False)
    wv = nc.declare_dram_parameter("wv_t", [C, C], BF16, isOutput=False)
    wcp = nc.declare_dram_parameter("wcproj_t", [C, C], BF16, isOutput=False)
    wf1 = nc.declare_dram_parameter("wfc1_t", [C, HID], BF16, isOutput=False)
    wf2 = nc.declare_dram_parameter("wfc2_t", [HID, C], BF16, isOutput=False)
    out = nc.declare_dram_parameter("out", [MYQ, C], F32, isOutput=True)

    qsel = None
    if comm:
        qsel = nc.declare_dram_parameter("qsel", [4], F32, isOutput=False)

    lng = lnb = bias = f1b = None
    if not ln_identity:
        lng = nc.declare_dram_parameter("ln_g", [3, C], F32, isOutput=False)
        lnb = nc.declare_dram_parameter("ln_b", [3, C], F32, isOutput=False)
    if not zero_bias:
        bias = nc.declare_dram_parameter("bias3", [3, C], F32, isOutput=False)
        f1b = nc.declare_dram_parameter("fc1_b", [HID], F32, isOutput=False)

    x1d = None
    if not comm:
        x1d = nc.dram_tensor("x1d", [S, C], F32)  # phase-1 output spill

    with contextlib.ExitStack() as st:
        tc = st.enter_context(tile.TileContext(nc))
        consts = st.enter_context(tc.tile_pool(name="consts", bufs=1))
        identb = consts.tile([128, 128], BF16)
        make_identity(nc, identb)
        eps = consts.tile([128, 1], F32)
        nc.vector.memset(eps, 1e-5)

        oh_bc = None
        if comm:
            oh_bc = consts.tile([128, 4], F32)
            qap = qsel[:]
            bc = bass.AP(tensor=qap.tensor, offset=qap.offset,
                         ap=[[0, 128]] + list(qap.ap))
            nc.gpsimd.dma_start(out=oh_bc[:], in_=bc)

        gbt = bbt = bias_bc = f1b_t = None
        if not ln_identity:
            gbt = consts.tile([128, 3, C], F32)
            bbt = consts.tile([128, 3, C], F32)
            for t, src in ((gbt, lng), (bbt, lnb)):
                bc = bass.AP(tensor=src.tensor, offset=src.offset,
                             ap=[[0, 128]] + list(src.ap))
                nc.gpsimd.dma_start(out=t[:], in_=bc)
        if not zero_bias:
            bias_bc = consts.tile([128, 3, C], F32)
            bc = bass.AP(tensor=bias.tensor, offset=bias.offset,
                         ap=[[0, 128]] + list(bias.ap))
            nc.gpsimd.dma_start(out=bias_bc[:], in_=bc)
            f1b_t = consts.tile([128, HK], F32)
            nc.gpsimd.dma_start(out=f1b_t[:], in_=f1b.rearrange("(a p) -> p a", p=128))

        def ln(pool, x_ap, h_ap, which):
            """layernorm over free dim C; x_ap/h_ap [128, C]"""
            stt = pool.tile([128, 3, 6], F32, tag="ln_st")
            for sg in range(3):
                nc.vector.bn_stats(out=stt[:, sg, :],
                                   in_=x_ap[:, sg * 256:(sg + 1) * 256])
            mv = pool.tile([128, 2], F32, tag="ln_mv")
            nc.vector.bn_aggr(out=mv[:], in_=stt[:])
            nm = pool.tile([128, 2], F32, tag="ln_nm")  # [neg-mean, rstd]
            nc.vector.tensor_scalar_mul(nm[:, 0:1], mv[:, 0:1], -1.0)
            nc.scalar.activation(nm[:, 1:2], mv[:, 1:2], AF.Sqrt, bias=eps[:])
            nc.vector.reciprocal(nm[:, 1:2], nm[:, 1:2])
            nc.vector.tensor_scalar(h_ap, x_ap, nm[:, 0:1], nm[:, 1:2],
                                    ALU.add, ALU.mult)
            if not ln_identity:
                nc.vector.tensor_mul(h_ap, h_ap, gbt[:, which, :])
                nc.vector.tensor_add(h_ap, h_ap, bbt[:, which, :])

        def transpose4(psp, dst_ap, src_fn, n=4):
            """n [128,128] bf16 blocks src_fn(i) -> dst_ap ([128, n*128])"""
            ps = psp.tile([128, n * 128], BF16, tag="tps")
            for i in range(n):
                nc.tensor.matmul(ps[:, i * 128:(i + 1) * 128], src_fn(i),
                                 identb[:], is_transpose=True)
            nc.any.tensor_copy(dst_ap, ps[:])

        def mm2(ps, w_ap, x_ap, kc, nkc):
            """one step of a dense PSUM accumulation chain"""
            nc.tensor.matmul(ps, w_ap, x_ap,
                             start=kc == 0, stop=kc == nkc - 1)

        def attn(hp, qkT_kq, vaug, o_sb, pools, kb_base, vh_off):
            """per-view self-attention for one head pair (phase 1)."""
            ps_qk, ps_acc, sbp = pools
            qk = ps_qk.tile([128, 2, 2, 256], F32, tag="qk")
            for par in range(2):
                for kb in range(2):
                    nc.tensor.matmul(
                        qk[:, par, kb, :],
                        qkT_kq[par * 64:(par + 1) * 64, 6 + hp,
                               vh_off + kb * 128: vh_off + (kb + 1) * 128],
                        qkT_kq[par * 64:(par + 1) * 64, hp, vh_off:vh_off + 256],
                        start=True, stop=True, tile_position=(par * 64, 0))
            probs = sbp.tile([128, 2, 2, 256], BF16, tag="probs")
            nc.scalar.activation(probs[:], qk[:], AF.Exp, scale=SCALE)
            acc = ps_acc.tile([128, 4, 128], F32, tag="acc")  # slot=qb*2+par
            # slot-major: accumulation groups in one PSUM bank must not overlap
            for par in range(2):
                for qb in range(2):
                    for kb in range(2):
                        nc.tensor.matmul(
                            acc[:, qb * 2 + par, 0:65],
                            probs[:, par, kb, qb * 128:(qb + 1) * 128],
                            vaug[:, kb_base + kb, hp * 2 + par, :],
                            start=kb == 0, stop=kb == 1)
            recip = sbp.tile([128, 4, 1], F32, tag="recip")
            nc.vector.reciprocal(recip[:], acc[:, :, 64:65])
            for qb in range(2):
                tb = (vh_off // 256) * 2 + qb
                dst = o_sb[:, tb, hp * 128:(hp + 1) * 128].rearrange(
                    "p (t d) -> p t d", t=2)
                nc.vector.tensor_mul(
                    dst, acc[:, qb * 2:qb * 2 + 2, 0:64],
                    recip[:, qb * 2:qb * 2 + 2, :].broadcast_to([128, 2, 64]))

        p23 = st.enter_context(tc.tile_pool(name="p23", bufs=1))
        x2 = p23.tile([128, 4, C], F32)

        x1g_in = x1g_out = None
        if comm:
            dram = st.enter_context(tc.tile_pool(name="dramp", bufs=1,
                                                 space="DRAM"))
            # one gather per own view so the first overlaps phase-1 tail
            # and the second overlaps 2a compute on the first half
            x1g_in = [dram.tile([256, C], BF16, name=f"x1g_in{h}")
                      for h in range(2)]
            x1g_out = [dram.tile([4, 256, C], BF16, name=f"x1g_out{h}")
                       for h in range(2)]

        # the big phase-2 tensors; opened before phase 1 only when phase 1
        # writes into them directly (non-comm)
        big_st = contextlib.ExitStack()

        def open_big():
            big = big_st.enter_context(tc.tile_pool(name="p2big", bufs=1))
            t = {}
            t["k2T"] = big.tile([128, CK, S], BF16, name="k2T")
            t["v2aug"] = big.tile([128, SK, 12, 65], BF16, name="v2aug")
            t["h2qT"] = big.tile([128, CK, MYQ], BF16, name="h2qT")
            t["q2T"] = big.tile([128, CK, MYQ], BF16, name="q2T")
            t["x1m"] = big.tile([128, 4, C], F32, name="x1m")
            t["o_sb2"] = big.tile([128, 4, C], BF16, name="o_sb2")
            nc.vector.memset(t["v2aug"][:, :, :, 64:65], 1.0)
            return t

        bigt = open_big() if not comm else None

        # ============ phase 1: per-view self-attention (own views) ============
        with tc.tile_pool(name="p1w", bufs=1) as p1w, \
             tc.tile_pool(name="p1b", bufs=1) as p1b, \
             tc.tile_pool(name="p1s", bufs=3) as p1s, \
             tc.tile_pool(name="ps_t", bufs=1, space="PSUM") as ps_t, \
             tc.tile_pool(name="ps_mm", bufs=2, space="PSUM") as ps_mm, \
             tc.tile_pool(name="ps_mb", bufs=1, space="PSUM") as ps_mb, \
             tc.tile_pool(name="ps_qk", bufs=1, space="PSUM") as ps_qk, \
             tc.tile_pool(name="ps_acc", bufs=2, space="PSUM") as ps_acc:

            wqkv_s = p1w.tile([128, CK, 3 * C], BF16)
            wproj_s = p1w.tile([128, CK, C], BF16)
            for kc in range(CK):
                nc.sync.dma_start(out=wqkv_s[:, kc, :], in_=wqkv[kc * 128:(kc + 1) * 128, :])
                nc.sync.dma_start(out=wproj_s[:, kc, :], in_=wproj[kc * 128:(kc + 1) * 128, :])

            for pair in range(npair):
                xv = p1b.tile([128, 4, C], F32, tag="xv")
                for tb in range(4):
                    r0 = pair * 512 + tb * 128
                    nc.sync.dma_start(out=xv[:, tb, :], in_=xb[r0:r0 + 128, :])
                h1 = p1b.tile([128, 4, C], BF16, tag="h1")
                for tb in range(4):
                    ln(p1s, xv[:, tb, :], h1[:, tb, :], 0)
                h1T = p1b.tile([128, CK, 512], BF16, tag="h1T")
                for kc in range(CK):
                    transpose4(ps_t, h1T[:, kc, :],
                               lambda tb, kc=kc: h1[:, tb, kc * 128:(kc + 1) * 128])

                qkT = p1b.tile([128, 12, 512], BF16, tag="qkT")
                for mo in range(12):
                    ps = ps_mm.tile([128, 512], F32, tag="mm")
                    for kc in range(CK):
                        nc.tensor.matmul(ps[:], wqkv_s[:, kc, mo * 128:(mo + 1) * 128],
                                         h1T[:, kc, :], start=kc == 0, stop=kc == CK - 1)
                    nc.any.tensor_copy(qkT[:, mo, :], ps[:])
                v1aug = p1b.tile([128, 4, 12, 65], BF16, tag="v1aug")
                nc.vector.memset(v1aug[:, :, :, 64:65], 1.0)
                for tb in range(4):
                    ps0 = ps_mm.tile([128, 512], F32, tag="mm")
                    ps1 = ps_mb.tile([128, 256], F32, tag="mb")
                    for kc in range(CK):
                        nc.tensor.matmul(ps0[:], h1T[:, kc, tb * 128:(tb + 1) * 128],
                                         wqkv_s[:, kc, 2 * C:2 * C + 512],
                                         start=kc == 0, stop=kc == CK - 1)
                        nc.tensor.matmul(ps1[:], h1T[:, kc, tb * 128:(tb + 1) * 128],
                                         wqkv_s[:, kc, 2 * C + 512:3 * C],
                                         start=kc == 0, stop=kc == CK - 1)
                    nc.any.tensor_copy(v1aug[:, tb, 0:8, 0:64],
                                       ps0[:].rearrange("p (h d) -> p h d", h=8))
                    nc.any.tensor_copy(v1aug[:, tb, 8:12, 0:64],
                                       ps1[:].rearrange("p (h d) -> p h d", h=4))

                o_sb = p1b.tile([128, 4, C], BF16, tag="osb")
                o1T = p1b.tile([128, 2, CK, 256], BF16, tag="o1T")
                x1f = p1b.tile([128, 4, C], F32, tag="x1f")
                x1b16 = p1b.tile([128, 4, C], BF16, tag="x1b16")
                for vh in range(2):
                    for hp in range(NP):
                        attn(hp, qkT, v1aug, o_sb,
                             (ps_qk, ps_acc, p1s), vh * 2, vh * 256)
                    for kc in range(CK):
                        transpose4(ps_t, o1T[:, vh, kc, :],
                                   lambda i, kc=kc, vh=vh:
                                   o_sb[:, vh * 2 + i, kc * 128:(kc + 1) * 128],
                                   n=2)
                    # proj + residual -> x1 (token-major), one view at a time
                    for mc in range(2):
                        tb = vh * 2 + mc
                        ps0 = ps_mm.tile([128, 512], F32, tag="mm")
                        ps1 = ps_mb.tile([128, 256], F32, tag="mb")
                        for kc in range(CK):
                            nc.tensor.matmul(ps0[:],
                                             o1T[:, vh, kc, mc * 128:(mc + 1) * 128],
                                             wproj_s[:, kc, 0:512],
                                             start=kc == 0, stop=kc == CK - 1)
                            nc.tensor.matmul(ps1[:],
                                             o1T[:, vh, kc, mc * 128:(mc + 1) * 128],
                                             wproj_s[:, kc, 512:768],
                                             start=kc == 0, stop=kc == CK - 1)
                        nc.vector.tensor_add(x1f[:, tb, 0:512], ps0[:], xv[:, tb, 0:512])
                        nc.vector.tensor_add(x1f[:, tb, 512:768], ps1[:],
                                             xv[:, tb, 512:768])
                        if not zero_bias:
                            nc.vector.tensor_add(x1f[:, tb, :], x1f[:, tb, :],
                                                 bias_bc[:, 0, :])
                        if comm:
                            nc.any.tensor_copy(x1b16[:, tb, :], x1f[:, tb, :])
                            nc.sync.dma_start(
                                out=x1g_in[vh][mc * 128:(mc + 1) * 128, :],
                                in_=x1b16[:, tb, :])
                        else:
                            r0 = pair * 512 + tb * 128
                            nc.sync.dma_start(out=x1d[r0:r0 + 128, :],
                                              in_=x1f[:, tb, :])
                    if comm:
                        # share this view's phase-1 output across the group
                        nc.gpsimd.collective_compute(
                            "AllGather", mybir.AluOpType.bypass,
                            replica_groups=RG,
                            ins=[x1g_in[vh][:].opt()],
                            outs=[x1g_out[vh][:].opt()])

        if comm:
            bigt = open_big()
        k2T, v2aug, h2qT, q2T = (bigt["k2T"], bigt["v2aug"],
                                 bigt["h2qT"], bigt["q2T"])
        x1m, o_sb2 = bigt["x1m"], bigt["o_sb2"]

        # ===== 2a: ln2 + k/v projections for all views, by gather half =====
        # group (h, j) covers global views (4j+h, 4j+2+h); for the comm build
        # those live in x1g_out[h] srcs (2j, 2j+1), so half 0 can be processed
        # while the second AllGather is still in flight.
        with tc.tile_pool(name="p2aw", bufs=1) as p2aw, \
             tc.tile_pool(name="p2ab", bufs=2) as p2ab, \
             tc.tile_pool(name="p2as", bufs=3) as p2as, \
             tc.tile_pool(name="ps2_t", bufs=2, space="PSUM") as ps2_t, \
             tc.tile_pool(name="ps2_mm", bufs=2, space="PSUM") as ps2_mm, \
             tc.tile_pool(name="ps2_mb", bufs=1, space="PSUM") as ps2_mb:
            wk_s = p2aw.tile([128, CK, C], BF16)
            wv_s = p2aw.tile([128, CK, C], BF16)
            wq_s = p2aw.tile([128, CK, C], BF16)
            for kc in range(CK):
                nc.sync.dma_start(out=wk_s[:, kc, :], in_=wk[kc * 128:(kc + 1) * 128, :])
                nc.sync.dma_start(out=wv_s[:, kc, :], in_=wv[kc * 128:(kc + 1) * 128, :])
                nc.sync.dma_start(out=wq_s[:, kc, :], in_=wq[kc * 128:(kc + 1) * 128, :])

            def process_group(h, j):
                va, vb = 4 * j + h, 4 * j + 2 + h
                if comm:
                    x1v = p2ab.tile([128, 4, C], BF16, tag="x1v")
                    for tb in range(4):
                        nc.sync.dma_start(
                            out=x1v[:, tb, :],
                            in_=x1g_out[h][2 * j + tb // 2,
                                           (tb % 2) * 128:(tb % 2) * 128 + 128, :])
                else:
                    x1v = p2ab.tile([128, 4, C], F32, tag="x1v")
                    for tb in range(4):
                        v = va if tb < 2 else vb
                        r0 = v * L + (tb % 2) * 128
                        nc.sync.dma_start(out=x1v[:, tb, :], in_=x1d[r0:r0 + 128, :])
                h2 = p2ab.tile([128, 4, C], BF16, tag="h2")
                for tb in range(4):
                    ln(p2as, x1v[:, tb, :], h2[:, tb, :], 1)
                h2T = p2ab.tile([128, CK, 512], BF16, tag="h2T")
                for kc in range(CK):
                    transpose4(ps2_t, h2T[:, kc, :],
                               lambda tb, kc=kc: h2[:, tb, kc * 128:(kc + 1) * 128])
                    if not comm:
                        src = h2T[:, kc, :].rearrange("p (v t) -> p v t", v=2)[:, :, 0:64]
                        nc.any.tensor_copy(h2qT[:, kc, va * 64:(va + 1) * 64],
                                           src[:, 0, :])
                        nc.any.tensor_copy(h2qT[:, kc, vb * 64:(vb + 1) * 64],
                                           src[:, 1, :])
                for mo in range(CK):
                    ps = ps2_mm.tile([128, 512], F32, tag="mm")
                    for kc in range(CK):
                        mm2(ps, wk_s[:, kc, mo * 128:(mo + 1) * 128],
                            h2T[:, kc, :], kc, CK)
                    nc.any.tensor_copy(k2T[:, mo, va * 256:(va + 1) * 256],
                                       ps[:, 0:256])
                    nc.any.tensor_copy(k2T[:, mo, vb * 256:(vb + 1) * 256],
                                       ps[:, 256:512])
                for tb in range(4):
                    v = va if tb < 2 else vb
                    ps0 = ps2_mm.tile([128, 512], F32, tag="mm")
                    ps1 = ps2_mb.tile([128, 256], F32, tag="mb")
                    for kc in range(CK):
                        mm2(ps0, h2T[:, kc, tb * 128:(tb + 1) * 128],
                            wv_s[:, kc, 0:512], kc, CK)
                        mm2(ps1, h2T[:, kc, tb * 128:(tb + 1) * 128],
                            wv_s[:, kc, 512:768], kc, CK)
                    vdst = v2aug[:, v * 2 + tb % 2]
                    nc.any.tensor_copy(vdst[:, 0:8, 0:64],
                                       ps0[:].rearrange("p (h d) -> p h d", h=8))
                    nc.any.tensor_copy(vdst[:, 8:12, 0:64],
                                       ps1[:].rearrange("p (h d) -> p h d", h=4))

            for j in range(2):
                process_group(0, j)

            if comm:
                # blend the four query-slice candidates with the one-hot qsel
                # (DMA+DVE only; overlaps the PE work of the second 2a half)
                xc = []
                for j in range(4):
                    xcj = p2as.tile([128, 4, C], BF16, tag=f"xc{j}", bufs=1)
                    for half in range(2):
                        gsrc = x1g_out[half][:]
                        src = bass.AP(
                            tensor=gsrc.tensor,
                            offset=gsrc.offset + j * 64 * C,
                            ap=[[C, 64], [256 * C, 4], [1, C]])
                        nc.sync.dma_start(out=xcj[half * 64:(half + 1) * 64, :, :],
                                          in_=src)
                    xc.append(xcj)
                nc.vector.tensor_scalar_mul(x1m[:], xc[0][:], oh_bc[:, 0:1])
                for j in range(1, 4):
                    nc.vector.scalar_tensor_tensor(
                        x1m[:], xc[j][:], oh_bc[:, j:j + 1], x1m[:],
                        op0=ALU.mult, op1=ALU.add)
                h2q = p2as.tile([128, 4, C], BF16, tag="h2q")
                for r in range(4):
                    ln(p2as, x1m[:, r, :], h2q[:, r, :], 1)
            else:
                for v in range(V):
                    nc.sync.dma_start(out=x1m[(v % 2) * 64:(v % 2) * 64 + 64, v // 2, :],
                                      in_=x1d[v * L: v * L + QS, :])

            for j in range(2):
                process_group(1, j)

            # ---- q projection for my 512 query tokens ----
            if comm:
                for kc in range(CK):
                    transpose4(ps2_t, h2qT[:, kc, :],
                               lambda r, kc=kc: h2q[:, r, kc * 128:(kc + 1) * 128])
            for mo in range(CK):
                ps = ps2_mm.tile([128, 512], F32, tag="mm")
                for kc in range(CK):
                    mm2(ps, wq_s[:, kc, mo * 128:(mo + 1) * 128],
                        h2qT[:, kc, :], kc, CK)
                nc.any.tensor_copy(q2T[:, mo, :], ps[:])

        # ============ 2b: block-causal attention over key prefixes ============
        with tc.tile_pool(name="p2bs", bufs=3) as p2bs, \
             tc.tile_pool(name="ps2b_qk", bufs=2, space="PSUM") as ps2b_qk, \
             tc.tile_pool(name="ps2b_acc", bufs=1, space="PSUM") as ps2b_acc:
            for hp in range(NP):
                for qbp in range(2):  # query blocks [qbp*256, qbp*256+256)
                    q0 = qbp * 256
                    ng = 4 * qbp + 4       # key views visible to this pair
                    # slot = lqb*2 + par, each slot in its own bank
                    acc = ps2b_acc.tile([128, 4, 512], F32, tag="acc")
                    for g in range(ng):
                        # local query window allowed for key view g
                        # (key views 0,1 are visible to every query)
                        qs = 0 if g < 2 else max(0, g * 64 - q0)
                        nq = 256 - qs
                        qk = ps2b_qk.tile([128, 2, 2, 256], F32, tag="qk")
                        for par in range(2):
                            for i in range(2):
                                kb = 2 * g + i
                                nc.tensor.matmul(
                                    qk[:, par, i, 0:nq],
                                    k2T[par * 64:(par + 1) * 64, hp,
                                        kb * 128:(kb + 1) * 128],
                                    q2T[par * 64:(par + 1) * 64, hp,
                                        q0 + qs:q0 + 256],
                                    start=True, stop=True,
                                    tile_position=(par * 64, 0))
                        probs = p2bs.tile([128, 2, 2, 256], BF16, tag="probs")
                        if qs % 128:
                            lqb0 = qs // 128
                            nc.vector.memset(probs[:, :, :, lqb0 * 128:qs], 0.0)
                        if qs:
                            nc.scalar.activation(probs[:, :, :, qs:256],
                                                 qk[:, :, :, 0:nq],
                                                 AF.Exp, scale=SCALE)
                        else:
                            nc.scalar.activation(probs[:], qk[:],
                                                 AF.Exp, scale=SCALE)
                        for par in range(2):
                            for lqb in range(qs // 128, 2):
                                last_g = 2 * (2 * qbp + lqb) + 1
                                if g > last_g:
                                    continue
                                for i in range(2):
                                    kb = 2 * g + i
                                    nc.tensor.matmul(
                                        acc[:, lqb * 2 + par, 0:65],
                                        probs[:, par, i,
                                              lqb * 128:(lqb + 1) * 128],
                                        v2aug[:, kb, hp * 2 + par, :],
                                        start=(g == 0 and i == 0),
                                        stop=(g == last_g and i == 1))
                    recip = p2bs.tile([128, 4, 1], F32, tag="recip")
                    nc.vector.reciprocal(recip[:], acc[:, :, 64:65])
                    for lqb in range(2):
                        dst = o_sb2[:, qbp * 2 + lqb,
                                    hp * 128:(hp + 1) * 128].rearrange(
                            "p (t d) -> p t d", t=2)
                        nc.vector.tensor_mul(
                            dst, acc[:, lqb * 2:lqb * 2 + 2, 0:64],
                            recip[:, lqb * 2:lqb * 2 + 2, :].broadcast_to(
                                [128, 2, 64]))

        # ============ 2c: cproj + residual ============
        with tc.tile_pool(name="p2cw", bufs=1) as p2cw, \
             tc.tile_pool(name="p2cs", bufs=2) as p2cs, \
             tc.tile_pool(name="ps2c_t", bufs=2, space="PSUM") as ps2c_t, \
             tc.tile_pool(name="ps2c_mm", bufs=2, space="PSUM") as ps2c_mm, \
             tc.tile_pool(name="ps2c_mb", bufs=2, space="PSUM") as ps2c_mb:
            wcp_s = p2cw.tile([128, CK, C], BF16)
            for kc in range(CK):
                nc.sync.dma_start(out=wcp_s[:, kc, :],
                                  in_=wcp[kc * 128:(kc + 1) * 128, :])
            o2T = p2cw.tile([128, CK, MYQ], BF16)
            for kc in range(CK):
                transpose4(ps2c_t, o2T[:, kc, :],
                           lambda tb, kc=kc: o_sb2[:, tb, kc * 128:(kc + 1) * 128])
            for tb in range(4):
                ps0 = ps2c_mm.tile([128, 512], F32, tag="mm")
                ps1 = ps2c_mb.tile([128, 256], F32, tag="mb")
                for kc in range(CK):
                    mm2(ps0, o2T[:, kc, tb * 128:(tb + 1) * 128],
                        wcp_s[:, kc, 0:512], kc, CK)
                    mm2(ps1, o2T[:, kc, tb * 128:(tb + 1) * 128],
                        wcp_s[:, kc, 512:768], kc, CK)
                nc.vector.tensor_add(x2[:, tb, 0:512], ps0[:], x1m[:, tb, 0:512])
                nc.vector.tensor_add(x2[:, tb, 512:768], ps1[:], x1m[:, tb, 512:768])
                if not zero_bias:
                    nc.vector.tensor_add(x2[:, tb, :], x2[:, tb, :],
                                         bias_bc[:, 1, :])

        big_st.close()

        # =================== phase 3: MLP ===================
        with tc.tile_pool(name="p3w", bufs=1) as p3w, \
             tc.tile_pool(name="p3one", bufs=1) as p3one, \
             tc.tile_pool(name="p3s", bufs=3) as p3s, \
             tc.tile_pool(name="ps3_t", bufs=2, space="PSUM") as ps3_t, \
             tc.tile_pool(name="ps3_mm", bufs=2, space="PSUM") as ps3_mm, \
             tc.tile_pool(name="ps3_mb", bufs=2, space="PSUM") as ps3_mb:
            wf1_s = p3w.tile([128, CK, HID], BF16)
            for kc in range(CK):
                nc.sync.dma_start(out=wf1_s[:, kc, :], in_=wf1[kc * 128:(kc + 1) * 128, :])
            wf2_s = p3w.tile([128, HK, C], BF16)
            for kc in range(HK):
                nc.sync.dma_start(out=wf2_s[:, kc, :], in_=wf2[kc * 128:(kc + 1) * 128, :])

            h3 = p3one.tile([128, 4, C], BF16)
            for tb in range(4):
                ln(p3s, x2[:, tb, :], h3[:, tb, :], 2)
            h3T = p3one.tile([128, CK, MYQ], BF16)
            for kc in range(CK):
                transpose4(ps3_t, h3T[:, kc, :],
                           lambda tb, kc=kc: h3[:, tb, kc * 128:(kc + 1) * 128])
            g1T = p3one.tile([128, HK, MYQ], BF16)
            for mo in range(HK):
                ps = ps3_mm.tile([128, MYQ], F32, tag="mm")
                for kc in range(CK):
                    mm2(ps, wf1_s[:, kc, mo * 128:(mo + 1) * 128],
                        h3T[:, kc, :], kc, CK)
                if sim_gelu:
                    # tanh-approx gelu from sim-supported ops (sim only)
                    xg = p3s.tile([128, MYQ], F32, tag="xg")
                    if zero_bias:
                        nc.any.tensor_copy(xg[:], ps[:])
                    else:
                        nc.scalar.activation(xg[:], ps[:], AF.Identity,
                                             bias=f1b_t[:, mo:mo + 1])
                    x2g = p3s.tile([128, MYQ], F32, tag="x2g")
                    nc.scalar.activation(x2g[:], xg[:], AF.Square)
                    nc.vector.tensor_scalar(x2g[:], x2g[:], 0.0356774081,
                                            0.7978845608, ALU.mult, ALU.add)
                    nc.vector.tensor_mul(x2g[:], x2g[:], xg[:])
                    nc.scalar.activation(x2g[:], x2g[:], AF.Tanh)
                    nc.vector.tensor_mul(x2g[:], x2g[:], xg[:])
                    nc.vector.tensor_add(x2g[:], x2g[:], xg[:])
                    nc.vector.tensor_scalar_mul(x2g[:], x2g[:], 0.5)
                    nc.any.tensor_copy(g1T[:, mo, :], x2g[:])
                elif zero_bias:
                    nc.scalar.activation(g1T[:, mo, :], ps[:], AF.Gelu)
                else:
                    nc.scalar.activation(g1T[:, mo, :], ps[:], AF.Gelu,
                                         bias=f1b_t[:, mo:mo + 1])
            for tb in range(4):
                ps0 = ps3_mm.tile([128, 512], F32, tag="mm")
                ps1 = ps3_mb.tile([128, 256], F32, tag="mb")
                for kc in range(HK):
                    mm2(ps0, g1T[:, kc, tb * 128:(tb + 1) * 128],
                        wf2_s[:, kc, 0:512], kc, HK)
                    mm2(ps1, g1T[:, kc, tb * 128:(tb + 1) * 128],
                        wf2_s[:, kc, 512:768], kc, HK)
                yo = p3s.tile([128, C], F32, tag="yo")
                nc.vector.tensor_add(yo[:, 0:512], ps0[:], x2[:, tb, 0:512])
                nc.vector.tensor_add(yo[:, 512:768], ps1[:], x2[:, tb, 512:768])
                if not zero_bias:
                    nc.vector.tensor_add(yo[:], yo[:], bias_bc[:, 2, :])
                nc.sync.dma_start(out=out[tb * 128:(tb + 1) * 128, :], in_=yo[:])

    nc.finalize()
    return nc


_CACHE = {}


def _get_nc(ln_identity, zero_bias, sim_gelu=False, comm=USE_COMM):
    key = (ln_identity, zero_bias, sim_gelu, comm)
    if key not in _CACHE:
        _CACHE[key] = _build(ln_identity, zero_bias, sim_gelu, comm)
    return _CACHE[key]


def _prep_inputs(inputs, comm=USE_COMM):
    x = np.asarray(inputs["x"], np.float32)          # [B, V, L, C]
    ln_identity = all(np.all(np.asarray(inputs[f"ln{i}_g"]) == 1.0)
                      and np.all(np.asarray(inputs[f"ln{i}_b"]) == 0.0)
                      for i in (1, 2, 3))
    zero_bias = all(np.all(np.asarray(inputs[k]) == 0.0)
                    for k in ("attn_proj_b", "cproj_b", "fc1_b", "fc2_b"))

    tr = lambda k: np.ascontiguousarray(
        np.asarray(inputs[k], np.float32).T).astype(ml_dtypes.bfloat16)
    wqkv_t, wproj_t = tr("qkv_w"), tr("attn_proj_w")
    wq_t, wk_t, wv_t, wcp_t = tr("q_w"), tr("k_w"), tr("v_w"), tr("cproj_w")
    wf1_t = tr("fc1_w")
    wf2_t = tr("fc2_w")

    in_maps = []
    for c in range(NCORES):
        b, g = divmod(c, G)
        if comm:
            xbp = np.ascontiguousarray(
                x[b, 2 * g:2 * g + 2].reshape(512, C))
        else:
            xbp = np.empty((S, C), np.float32)
            for v in range(V):
                xv = x[b, v]
                xbp[v * L: v * L + QS] = xv[g * QS:(g + 1) * QS]
                xbp[v * L + QS: v * L + QS + g * QS] = xv[0: g * QS]
                xbp[v * L + QS + g * QS: (v + 1) * L] = xv[(g + 1) * QS:]
        m = {"xb": xbp, "wqkv_t": wqkv_t, "wproj_t": wproj_t, "wq_t": wq_t,
             "wk_t": wk_t, "wv_t": wv_t, "wcproj_t": wcp_t, "wfc1_t": wf1_t,
             "wfc2_t": wf2_t}
        if comm:
            m["qsel"] = np.eye(4, dtype=np.float32)[g]
        if not ln_identity:
            m["ln_g"] = np.stack([np.asarray(inputs[f"ln{i}_g"], np.float32)
                                  for i in (1, 2, 3)])
            m["ln_b"] = np.stack([np.asarray(inputs[f"ln{i}_b"], np.float32)
                                  for i in (1, 2, 3)])
        if not zero_bias:
            m["bias3"] = np.stack([np.asarray(inputs["attn_proj_b"], np.float32),
                                   np.asarray(inputs["cproj_b"], np.float32),
                                   np.asarray(inputs["fc2_b"], np.float32)])
            m["fc1_b"] = np.asarray(inputs["fc1_b"], np.float32)
        in_maps.append(m)
    return in_maps, ln_identity, zero_bias


def _assemble(results):
    out = np.empty((B, V, L, C), np.float32)
    for c in range(NCORES):
        b, g = divmod(c, G)
        oc = np.asarray(results[c]["out"])
        for v in range(V):
            out[b, v, g * QS:(g + 1) * QS] = oc[v * QS:(v + 1) * QS]
    return out


def kernel(**inputs):
    in_maps, ln_identity, zero_bias = _prep_inputs(inputs)
    nc = _get_nc(ln_identity, zero_bias)
    res = run_bass_kernel_spmd(nc, in_maps, core_ids=list(range(NCORES)))
    return _assemble(res.results)
